# revision 1
# baseline (speedup 1.0000x reference)
"""Multi-head causal attention (B=2, S=2048, D=1024, H=16) on 8 TRN2 NeuronCores.

Sharding: tensor-parallel over heads x data-parallel over batch.
Core c handles batch b = c // 4 and head group g = c % 4 (heads 4g..4g+3),
i.e. a [2048, 256] slice of the output.

Per-core kernel (all fp32 data, matmuls in float32r: 1 cycle/row at
moving-dim >= 256 vs 4 for plain fp32):
  - Q^T/K^T projections produce d-major [256, 2048] activations directly
    (lhsT = W column slice, rhs = host-pretransposed x^T), k-outer so each
    weight tile stays stationary for 4 matmuls.
  - V projection produces k-major V' tiles [128, 4*65] with a ones column
    per head, so the PV matmul also yields softmax denominators; V's bias
    rides in as a K=1 ones x bias-row matmul.
  - Scores are computed transposed (S^T = K @ Q^T, [k partitions, q free]):
    softmax needs no per-tile transposes. Head pairs share one 128-row
    Q^T/K^T tile; their two K=64 score matmuls go to distinct PE row groups
    (tile_position (0,0)/(64,0)) and distinct PSUM banks of one [128,1024]
    tile, running concurrently, and share a single strided exp on ScalarE
    (halving ScalarE's ~352-cycle fixed cost per op).
  - Causal masking adds -1e30 triangles on VectorE before the exp; windows
    are exact (qoff = 128*i on diagonal-region tiles).
  - Normalization: 1/s = exp(-ln(s)) on ScalarE (plain DVE reciprocal runs
    ~7 cycles/element), broadcast across partitions by a K=1 ones matmul,
    then one VectorE multiply.
  - Output is written d-major [256, 2048] and transposed on the host.
"""

import os
import sys

import numpy as np

for _p in ("/opt/trn_rl_repo", "/root/.axon_site/_ro/trn_rl_repo"):
    if os.path.isdir(_p) and _p not in sys.path:
        sys.path.insert(0, _p)

B, S, D, H = 2, 2048, 1024, 16
N_CORES = 8
HEADS_PER_CORE = 4
DH = D // H  # 64
DCORE = HEADS_PER_CORE * DH  # 256
KT = D // 128  # 8 contraction tiles for the projections
ST = S // 128  # 16 sequence tiles
QB = 512  # q block width
NEG = -1.0e30

_CACHE = {}


def _split_multi_waits(nc, max_waits=1):
    """This walrus build rejects instructions carrying more than one
    semaphore wait; hoist extras onto preceding NoOps on the same engine."""
    import bass_rust as _br

    n = 0
    for fn in nc.m.functions:
        for bb in fn.blocks:
            insts = list(bb.instructions)
            new = []
            changed = False
            for inst in insts:
                si = getattr(inst, "sync_info", None)
                ow = list(si.on_wait) if si is not None else []
                if len(ow) > max_waits:
                    changed = True
                    for w in ow[:-max_waits]:
                        n += 1
                        new.append(
                            _br.InstNoOp(
                                name=f"I-ws{n}",
                                engine=inst.engine,
                                ins=[],
                                outs=[],
                                sync_info=_br.SyncInfo(on_wait=[w], on_update=[]),
                            )
                        )
                    si.on_wait = ow[-max_waits:]
                    inst.sync_info = si
                new.append(inst)
            if changed:
                bb.instructions = new


def build_module(repeat=1, hw_loop=False):
    import contextlib

    import concourse.bass as bass
    import concourse.mybir as mybir
    from concourse.tile import TileContext

    F32 = mybir.dt.float32
    F32R = mybir.dt.float32r
    AF = mybir.ActivationFunctionType

    nc = bass.Bass("TRN2", target_bir_lowering=False, debug=False, num_devices=N_CORES)

    xT_in = nc.declare_dram_parameter("xT", [D, S], F32, isOutput=False)
    wq_in = nc.declare_dram_parameter("wq", [D, DCORE], F32, isOutput=False)
    wk_in = nc.declare_dram_parameter("wk", [D, DCORE], F32, isOutput=False)
    wv_in = nc.declare_dram_parameter("wv", [D, DCORE], F32, isOutput=False)
    bq_in = nc.declare_dram_parameter("bq", [DCORE], F32, isOutput=False)
    bk_in = nc.declare_dram_parameter("bk", [DCORE], F32, isOutput=False)
    bv_in = nc.declare_dram_parameter("bv", [DCORE], F32, isOutput=False)
    tri_in = nc.declare_dram_parameter("tri", [128, 256], F32, isOutput=False)
    ones_in = nc.declare_dram_parameter("ones", [128, 4], F32, isOutput=False)
    outT = nc.declare_dram_parameter("outT", [DCORE, S], F32, isOutput=True)

    with TileContext(nc) as tc:
        with (
            tc.tile_pool(name="persist", bufs=1) as pp,
            tc.tile_pool(name="work", bufs=4) as wp,
            tc.tile_pool(name="outp", bufs=3) as op,
            tc.tile_pool(name="mm_ps", bufs=2, space="PSUM") as mm_ps,
            tc.tile_pool(name="att_ps", bufs=4, space="PSUM") as att_ps,
        ):
            # ---- constant / persistent tiles -------------------------------
            trid = pp.tile([128, 256], F32, tag="trid")
            nc.sync.dma_start(trid[:], tri_in[:])
            onesr = pp.tile([1, 128], F32R, tag="onesr")  # K=1 matmul lhsT
            nc.sync.dma_start(
                onesr[:], ones_in[:, 0:1].rearrange("p a -> a p").bitcast(F32R)
            )
            bvrow = pp.tile([1, DCORE], F32R, tag="bvrow")
            nc.sync.dma_start(
                bvrow[:], bv_in[:].rearrange("(a b) -> a b", a=1).bitcast(F32R)
            )
            bqc = pp.tile([128, 2], F32, tag="bqc")
            nc.sync.dma_start(bqc[:], bq_in[:].rearrange("(m p) -> p m", p=128))
            bkc = pp.tile([128, 2], F32, tag="bkc")
            nc.sync.dma_start(bkc[:], bk_in[:].rearrange("(m p) -> p m", p=128))

            wq = []
            wk = []
            wv = []
            for k in range(KT):
                for name, lst, src in (("wq", wq, wq_in), ("wk", wk, wk_in), ("wv", wv, wv_in)):
                    t = pp.tile([128, DCORE], F32R, tag=f"{name}{k}")
                    nc.sync.dma_start(
                        t[:], src[128 * k : 128 * (k + 1), :].bitcast(F32R)
                    )
                    lst.append(t)
            # ---- warmup during the x DMA window: ~4.5us of dummy matmuls
            # ramps the PE HAM clock gate to 2.4 GHz, and one exp/ln pulls
            # the activation table load off the critical path --------------
            warm_ps = mm_ps.tile([128, 2 * QB], F32, tag="mm", name="warm_ps")
            for _w in range(42):
                nc.tensor.matmul(
                    warm_ps[:, 0:DCORE], onesr[:], bvrow[:], start=True, stop=True
                )
            warm_o = wp.tile([1, 128], F32, tag="lns", name="warm_o")
            nc.scalar.activation(warm_o[:], onesr[:].bitcast(F32), AF.Exp)
            nc.scalar.activation(warm_o[:], warm_o[:], AF.Ln)

            # x^T tiles, loaded in [128, QB] slices n-major so the first
            # projection blocks can start after ~1/4 of x has landed
            xt = [pp.tile([128, S], F32R, tag=f"xt{k}", name=f"xt{k}") for k in range(KT)]
            for n in range(S // QB):
                for k in range(KT):
                    nc.sync.dma_start(
                        xt[k][:, QB * n : QB * (n + 1)],
                        xT_in[128 * k : 128 * (k + 1), QB * n : QB * (n + 1)].bitcast(
                            F32R
                        ),
                    )

            if hw_loop and repeat > 1:
                rep_iter = [0]
                rep_ctx = tc.For_i(0, repeat, 1)
            else:
                rep_iter = range(repeat)
                rep_ctx = contextlib.nullcontext()
            with rep_ctx:
              for _rep in rep_iter:
                # ---- V' projection first, n-major (overlaps the x load) ----
                qT = [pp.tile([128, S], F32R, tag=f"qT{m}", name=f"qT{m}") for m in range(2)]
                kTt = [pp.tile([128, S], F32R, tag=f"kT{m}", name=f"kT{m}") for m in range(2)]
                vp = [pp.tile([128, 4 * 65], F32R, tag=f"vp{s}", name=f"vp{s}") for s in range(ST)]
                for s in range(ST):
                    dst = vp[s]
                    nc.sync.dma_start(
                        dst[:].rearrange("p (h c) -> p h c", c=65)[:, :, 64:65],
                        ones_in[:].rearrange("p (h c) -> p h c", c=1).bitcast(F32R),
                    )
                    ps = mm_ps.tile([128, DCORE], F32, tag="mm")
                    for k in range(KT):
                        nc.tensor.matmul(
                            ps[:],
                            xt[k][:, 128 * s : 128 * (s + 1)],
                            wv[k][:],
                            start=(k == 0),
                            stop=False,
                        )
                    nc.tensor.matmul(ps[:], onesr[:], bvrow[:], start=False, stop=True)
                    nc.vector.tensor_copy(
                        dst[:].rearrange("p (h c) -> p h c", c=65)[:, :, 0:64],
                        ps[:].rearrange("p (h c) -> p h c", c=64),
                    )

                # ---- Q^T/K^T projections, k-outer so each weight tile stays
                # stationary for 4 consecutive matmuls (att_ps banks are idle
                # here and hold the n-block accumulators) ---------------------
                for lst, w, bias in ((qT, wq, bqc), (kTt, wk, bkc)):
                    for m in range(2):
                        accs = [
                            att_ps.tile([128, QB], F32, tag="att", name=f"acc{n}")
                            for n in range(S // QB)
                        ]
                        for k in range(KT):
                            for n in range(S // QB):
                                nc.tensor.matmul(
                                    accs[n][:],
                                    w[k][:, 128 * m : 128 * (m + 1)],
                                    xt[k][:, QB * n : QB * (n + 1)],
                                    start=(k == 0),
                                    stop=(k == KT - 1),
                                )
                        for n in range(S // QB):
                            nc.vector.tensor_scalar_add(
                                lst[m][:, QB * n : QB * (n + 1)],
                                accs[n][:],
                                bias[:, m : m + 1],
                            )

                # ---- attention: head pairs share the 128-row Q^T/K^T tiles.
                # Both heads' K=64 score matmuls go to distinct PE row groups
                # (tile_position) and distinct halves of one [128,1024] PSUM
                # tile, so they run concurrently and share one exp. j-blocks
                # are processed two at a time, t-outer, so each V' head slice
                # stays stationary for two consecutive PV matmuls ------------
                def normalize(ap, h, j):
                    # 1/s = exp(-ln(s)) on ScalarE: the plain DVE reciprocal
                    # runs at ~7 cycles/element
                    lns = wp.tile([1, QB], F32, tag="lns", name="lns")
                    nc.scalar.activation(lns[:], ap[64:65, :], AF.Ln)
                    rrow = wp.tile([1, QB], F32R, tag="rrow", name="rrow")
                    nc.scalar.activation(rrow[:], lns[:], AF.Exp, scale=-1.0)
                    rbp = mm_ps.tile([64, QB], F32, tag="mm", name="rbp")
                    nc.tensor.matmul(
                        rbp[:], onesr[:, 0:64], rrow[:], start=True, stop=True
                    )
                    rb = wp.tile([64, QB], F32, tag="rb", name="rb")
                    nc.vector.tensor_copy(rb[:], rbp[:])
                    att = op.tile([64, QB], F32, tag="att_out", name="att")
                    nc.vector.tensor_mul(att[:], ap[0:64, :], rb[:])
                    nc.sync.dma_start(
                        outT[64 * h : 64 * (h + 1), QB * j : QB * (j + 1)], att[:]
                    )

                for hp in range(2):
                    hA, hB = 2 * hp, 2 * hp + 1
                    qTm, kTm = qT[hp], kTt[hp]
                    for jp in range(2):
                        js = (2 * jp, 2 * jp + 1)
                        aps = {}
                        for j in js:
                            for h in (hA, hB):
                                aps[(h, j)] = att_ps.tile(
                                    [128, QB], F32, tag="att", name=f"aps{h}_{j}"
                                )
                        for t in range(8 * jp + 8):
                            active = [j for j in js if t <= 4 * j + 3]
                            pts = {}
                            for j in active:
                                i = t - 4 * j  # >= 0 on diagonal-region tiles
                                qoff = 128 * max(i, 0)
                                qwin = slice(QB * j + qoff, QB * (j + 1))
                                ktile = slice(128 * t, 128 * (t + 1))
                                sps = mm_ps.tile(
                                    [128, 2 * QB], F32, tag="mm", name="sps"
                                )
                                nc.tensor.matmul(
                                    sps[:, qoff:QB],
                                    kTm[0:64, ktile],
                                    qTm[0:64, qwin],
                                    start=True,
                                    stop=True,
                                    tile_position=(0, 0),
                                )
                                nc.tensor.matmul(
                                    sps[:, QB + qoff : 2 * QB],
                                    kTm[64:128, ktile],
                                    qTm[64:128, qwin],
                                    start=True,
                                    stop=True,
                                    tile_position=(64, 0),
                                )
                                spsv = sps[:].rearrange("p (two c) -> p two c", two=2)
                                if i >= 0:
                                    nc.vector.tensor_add(
                                        spsv[:, :, qoff : qoff + 128],
                                        spsv[:, :, qoff : qoff + 128],
                                        trid[:].rearrange("p (two c) -> p two c", two=2),
                                    )
                                pt = wp.tile([128, 2 * QB], F32R, tag="pt")
                                ptv = pt[:].rearrange("p (two c) -> p two c", two=2)
                                nc.scalar.activation(
                                    ptv[:, :, qoff:QB],
                                    spsv[:, :, qoff:QB],
                                    AF.Exp,
                                    scale=float(1.0 / np.sqrt(DH)),
                                )
                                pts[j] = (pt, qoff)
                            for h, off in ((hA, 0), (hB, QB)):
                                for j in active:
                                    pt, qoff = pts[j]
                                    nc.tensor.matmul(
                                        aps[(h, j)][0:65, qoff:QB],
                                        vp[t][:, 65 * h : 65 * h + 65],
                                        pt[:, off + qoff : off + QB],
                                        start=(t == 0),
                                        stop=(t == 4 * j + 3),
                                    )
                            for j in active:
                                if t == 4 * j + 3:
                                    normalize(aps[(hA, j)], hA, j)
                                    normalize(aps[(hB, j)], hB, j)

    _split_multi_waits(nc)
    return nc


def _get_runner():
    if "nc" not in _CACHE:
        _CACHE["nc"] = build_module()
    return _CACHE["nc"]


def _make_in_maps(x, Wq, bq, Wk, bk, Wv, bv):
    x = np.asarray(x, dtype=np.float32)
    Wq = np.asarray(Wq, dtype=np.float32)
    Wk = np.asarray(Wk, dtype=np.float32)
    Wv = np.asarray(Wv, dtype=np.float32)
    bq = np.asarray(bq, dtype=np.float32)
    bk = np.asarray(bk, dtype=np.float32)
    bv = np.asarray(bv, dtype=np.float32)

    kp = np.arange(128)[:, None]
    qf = np.arange(128)[None, :]
    tri = np.where(kp <= qf, 0.0, NEG).astype(np.float32)
    trid = np.concatenate([tri, tri], axis=1)
    ones = np.ones((128, 4), np.float32)

    xTs = [np.ascontiguousarray(x[b].T) for b in range(B)]
    in_maps = []
    for c in range(N_CORES):
        b = c // 4
        g = c % 4
        sl = slice(DCORE * g, DCORE * (g + 1))
        in_maps.append(
            {
                "xT": xTs[b],
                "wq": np.ascontiguousarray(Wq[:, sl]),
                "wk": np.ascontiguousarray(Wk[:, sl]),
                "wv": np.ascontiguousarray(Wv[:, sl]),
                "bq": np.ascontiguousarray(bq[sl]),
                "bk": np.ascontiguousarray(bk[sl]),
                "bv": np.ascontiguousarray(bv[sl]),
                "tri": trid,
                "ones": ones,
            }
        )
    return in_maps


def kernel(x, Wq, bq, Wk, bk, Wv, bv):
    from concourse.bass_utils import run_bass_kernel_spmd

    nc = _get_runner()
    in_maps = _make_in_maps(x, Wq, bq, Wk, bk, Wv, bv)
    res = run_bass_kernel_spmd(nc, in_maps, list(range(N_CORES)))
    out = np.empty((B, S, D), dtype=np.float32)
    for c in range(N_CORES):
        b = c // 4
        g = c % 4
        out[b, :, DCORE * g : DCORE * (g + 1)] = res.results[c]["outT"].T
    return out



# revision 13
# speedup vs baseline: 1.1980x; 1.1980x over previous
"""Multi-head causal attention (B=2, S=2048, D=1024, H=16) on 8 TRN2 NeuronCores.

Sharding: tensor-parallel over heads x data-parallel over batch.
Core c handles batch b = c // 4 and head group g = c % 4 (heads 4g..4g+3),
i.e. a [2048, 256] slice of the output.

Per-core kernel, bf16 data path (PSUM accumulation stays fp32):
  - x, W, Q^T, K^T, V', probabilities all bf16: matmuls run 1 cycle/row at
    any width (fp32r needs moving-dim >= 256), DMA bytes halve, and the
    narrow diagonal-tile matmuls stop paying the 4x fp32 penalty.
  - Q^T/K^T projections produce d-major [256, 2048] activations directly
    (lhsT = W column slice, rhs = host-pretransposed x^T) as single
    [128,1024]-out matmuls per (m, half, k): one PSUM tile holds two
    512-blocks, one bias-add per kilocolumn.
  - V' is k-major [128, 4*65] with a ones column per head (denominators
    ride the PV matmul); ones columns via gpsimd memset, V bias via a
    K=1 ones x bias-row matmul.
  - Scores are computed transposed (S^T = K @ Q^T): softmax needs no
    transposes. A head pair shares one 128-row Q^T/K^T tile; the two K=64
    score matmuls go to distinct PE row groups (tile_position (0,0)/(64,0))
    and the two halves of one [128,1024] PSUM tile, sharing a single
    strided exp on ScalarE.
  - Normalization: reciprocal of the denominator row on VectorE (594ns,
    off the critical ScalarE), broadcast across partitions by a K=1 ones
    matmul, one VectorE multiply reading both PSUM operands.
  - Output is written d-major bf16 [256, 2048] and transposed on the host.
"""

import os
import sys

import numpy as np

for _p in ("/opt/trn_rl_repo", "/root/.axon_site/_ro/trn_rl_repo"):
    if os.path.isdir(_p) and _p not in sys.path:
        sys.path.insert(0, _p)

B, S, D, H = 2, 2048, 1024, 16
N_CORES = 8
HEADS_PER_CORE = 4
DH = D // H  # 64
DCORE = HEADS_PER_CORE * DH  # 256
KT = D // 128  # 8 contraction tiles for the projections
ST = S // 128  # 16 sequence tiles
QB = 512  # q block width
NEG = -1.0e30

_CACHE = {}


def _split_multi_waits(nc, max_waits=1):
    """This walrus build rejects instructions carrying more than one
    semaphore wait; hoist extras onto preceding NoOps on the same engine."""
    import bass_rust as _br

    n = 0
    for fn in nc.m.functions:
        for bb in fn.blocks:
            insts = list(bb.instructions)
            new = []
            changed = False
            for inst in insts:
                si = getattr(inst, "sync_info", None)
                ow = list(si.on_wait) if si is not None else []
                if len(ow) > max_waits:
                    changed = True
                    for w in ow[:-max_waits]:
                        n += 1
                        new.append(
                            _br.InstNoOp(
                                name=f"I-ws{n}",
                                engine=inst.engine,
                                ins=[],
                                outs=[],
                                sync_info=_br.SyncInfo(on_wait=[w], on_update=[]),
                            )
                        )
                    si.on_wait = ow[-max_waits:]
                    inst.sync_info = si
                new.append(inst)
            if changed:
                bb.instructions = new


def build_module(repeat=1, hw_loop=False):
    import contextlib

    import concourse.bass as bass
    import concourse.mybir as mybir
    from concourse.tile import TileContext

    F32 = mybir.dt.float32
    F32R = mybir.dt.float32r
    BF16 = mybir.dt.bfloat16
    AF = mybir.ActivationFunctionType

    nc = bass.Bass("TRN2", target_bir_lowering=False, debug=False, num_devices=N_CORES)

    xT_in = nc.declare_dram_parameter("xT", [D, S], BF16, isOutput=False)
    wq_in = nc.declare_dram_parameter("wq", [D, DCORE], BF16, isOutput=False)
    wk_in = nc.declare_dram_parameter("wk", [D, DCORE], BF16, isOutput=False)
    wv_in = nc.declare_dram_parameter("wv", [D, DCORE], BF16, isOutput=False)
    bq_in = nc.declare_dram_parameter("bq", [DCORE], F32, isOutput=False)
    bk_in = nc.declare_dram_parameter("bk", [DCORE], F32, isOutput=False)
    bv_in = nc.declare_dram_parameter("bv", [DCORE], F32, isOutput=False)
    tri_in = nc.declare_dram_parameter("tri", [128, 256], F32, isOutput=False)
    ones_in = nc.declare_dram_parameter("ones", [128, 1], F32, isOutput=False)
    outT = nc.declare_dram_parameter("outT", [DCORE, S], F32, isOutput=True)

    with TileContext(nc) as tc:
        with (
            tc.tile_pool(name="persist", bufs=1) as pp,
            tc.tile_pool(name="work", bufs=4) as wp,
            tc.tile_pool(name="outp", bufs=3) as op,
            tc.tile_pool(name="mm_ps", bufs=2, space="PSUM") as mm_ps,
            tc.tile_pool(name="att_ps", bufs=2, space="PSUM") as att_ps,
        ):
            # ---- constant / persistent tiles -------------------------------
            trid = pp.tile([128, 256], F32, tag="trid")
            nc.sync.dma_start(trid[:], tri_in[:])
            onesr = pp.tile([1, 128], F32R, tag="onesr")  # K=1 matmul lhsT
            nc.sync.dma_start(
                onesr[:], ones_in[:, 0:1].rearrange("p a -> a p").bitcast(F32R)
            )
            bvrow = pp.tile([1, DCORE], F32R, tag="bvrow")
            nc.sync.dma_start(
                bvrow[:], bv_in[:].rearrange("(a b) -> a b", a=1).bitcast(F32R)
            )
            bqc = pp.tile([128, 2], F32, tag="bqc")
            nc.sync.dma_start(bqc[:], bq_in[:].rearrange("(m p) -> p m", p=128))
            bkc = pp.tile([128, 2], F32, tag="bkc")
            nc.sync.dma_start(bkc[:], bk_in[:].rearrange("(m p) -> p m", p=128))

            wq = []
            wk = []
            wv = []
            for k in range(KT):
                for name, lst, src in (("wq", wq, wq_in), ("wk", wk, wk_in), ("wv", wv, wv_in)):
                    t = pp.tile([128, DCORE], BF16, tag=f"{name}{k}")
                    nc.sync.dma_start(t[:], src[128 * k : 128 * (k + 1), :])
                    lst.append(t)
            # ---- warmup during the x DMA window: dummy matmuls ramp the PE
            # HAM clock gate to 2.4 GHz; one exp pulls the activation table
            # load off the critical path ------------------------------------
            warm_ps = mm_ps.tile([128, 2 * QB], F32, tag="mm", name="warm_ps")
            for _w in range(42):
                nc.tensor.matmul(
                    warm_ps[:, 0:DCORE], onesr[:], bvrow[:], start=True, stop=True
                )
            warm_o = wp.tile([1, 128], F32, tag="warm", name="warm_o")
            nc.scalar.activation(warm_o[:], onesr[:].bitcast(F32), AF.Exp)

            # x^T tiles, loaded in [128, QB] slices n-major so the first
            # projection blocks can start after ~1/4 of x has landed
            xt = [pp.tile([128, S], BF16, tag=f"xt{k}", name=f"xt{k}") for k in range(KT)]
            for n in range(S // QB):
                for k in range(KT):
                    nc.sync.dma_start(
                        xt[k][:, QB * n : QB * (n + 1)],
                        xT_in[128 * k : 128 * (k + 1), QB * n : QB * (n + 1)],
                    )

            qT = [pp.tile([128, S], BF16, tag=f"qT{m}", name=f"qT{m}") for m in range(2)]
            kTt = [pp.tile([128, S], BF16, tag=f"kT{m}", name=f"kT{m}") for m in range(2)]
            vp = [pp.tile([128, 4 * 65], BF16, tag=f"vp{s}", name=f"vp{s}") for s in range(ST)]

            if True:
                def v_chain(s):
                    # one V' sequence tile: k-major [128, 4*65] with a ones
                    # column per head (PV then also yields softmax denoms)
                    dst = vp[s]
                    nc.gpsimd.memset(
                        dst[:].rearrange("p (h c) -> p h c", c=65)[:, :, 64:65], 1.0
                    )
                    ps = mm_ps.tile([128, DCORE], F32, tag="mm", name=f"vps{s}")
                    for k in range(KT):
                        nc.tensor.matmul(
                            ps[:],
                            xt[k][:, 128 * s : 128 * (s + 1)],
                            wv[k][:],
                            start=(k == 0),
                            stop=False,
                        )
                    nc.tensor.matmul(ps[:], onesr[:], bvrow[:], start=False, stop=True)
                    nc.vector.tensor_copy(
                        dst[:].rearrange("p (h c) -> p h c", c=65)[:, :, 0:64],
                        ps[:].rearrange("p (h c) -> p h c", c=64),
                    )

                def qk_chain(lst, w, bias, m, half):
                    # one [128,1024]-wide accumulation chain of a Q/K proj;
                    # matmul outputs may not cross a PSUM bank, so each k
                    # contributes two 512-wide matmuls (weights stationary)
                    acc = mm_ps.tile([128, 2 * QB], F32, tag="mm", name="acc")
                    for k in range(KT):
                        for half2 in range(2):
                            nc.tensor.matmul(
                                acc[:, QB * half2 : QB * (half2 + 1)],
                                w[k][:, 128 * m : 128 * (m + 1)],
                                xt[k][
                                    :,
                                    2 * QB * half + QB * half2 : 2 * QB * half
                                    + QB * (half2 + 1),
                                ],
                                start=(k == 0),
                                stop=(k == KT - 1),
                            )
                    nc.vector.tensor_scalar_add(
                        lst[m][:, 2 * QB * half : 2 * QB * (half + 1)],
                        acc[:],
                        bias[:, m : m + 1],
                    )

            # ---- lead-in (once): V' s0-3 + Q/K m0 half0 unlock the first two
            # attention blocks; in the repeat loop these chains for the NEXT
            # iteration are emitted as fillers in the tail blocks, so the
            # steady-state body starts its attention immediately ------------
            v_chain(0)
            v_chain(1)
            v_chain(2)
            v_chain(3)
            qk_chain(qT, wq, bqc, 0, 0)
            qk_chain(kTt, wk, bkc, 0, 0)

            if hw_loop and repeat > 1:
                rep_iter = [0]
                rep_ctx = tc.For_i(0, repeat, 1)
            else:
                rep_iter = range(repeat)
                rep_ctx = contextlib.nullcontext()
            with rep_ctx:
              for _rep in rep_iter:
                # filler chains woven between t-iterations (one per two t's),
                # ordered so each completes before its consumer block; the
                # tail blocks carry the next iteration's lead-in chains
                per_block_fillers = {
                    0: [
                        lambda: v_chain(4),
                        lambda: qk_chain(qT, wq, bqc, 1, 0),
                    ],
                    1: [
                        lambda: v_chain(5),
                        lambda: qk_chain(kTt, wk, bkc, 1, 0),
                        lambda: v_chain(6),
                        lambda: v_chain(7),
                    ],
                    2: [
                        lambda: qk_chain(qT, wq, bqc, 0, 1),
                        lambda: qk_chain(kTt, wk, bkc, 0, 1),
                    ],
                    3: [
                        lambda: v_chain(8),
                        lambda: v_chain(9),
                        lambda: v_chain(10),
                        lambda: v_chain(11),
                    ],
                    4: [
                        lambda: qk_chain(qT, wq, bqc, 1, 1),
                        lambda: qk_chain(kTt, wk, bkc, 1, 1),
                        lambda: v_chain(12),
                        lambda: v_chain(13),
                        lambda: v_chain(14),
                        lambda: v_chain(15),
                    ],
                    6: [
                        lambda: qk_chain(qT, wq, bqc, 0, 0),
                        lambda: qk_chain(kTt, wk, bkc, 0, 0),
                    ],
                    7: [
                        lambda: v_chain(0),
                        lambda: v_chain(1),
                        lambda: v_chain(2),
                        lambda: v_chain(3),
                    ],
                }

                # ---- attention, software-pipelined emission: PV lags one
                # t-iteration behind its exp so the next scores never sit
                # behind a stalled PV in the in-order PE queue; normalize is
                # deferred two t-iterations into the following block --------
                blocks = [(0, 0), (0, 1), (1, 0), (1, 1), (0, 2), (0, 3), (1, 2), (1, 3)]
                prev_pv = None  # (aps, hA, hB, pt, qoff, t, last)
                pending_norm = None  # (aps, hA, hB, j)

                def emit_pv():
                    nonlocal prev_pv
                    if prev_pv is None:
                        return
                    aps, hA, hB, pt, qoff, t, last = prev_pv
                    for h, off in ((hA, 0), (hB, QB)):
                        nc.tensor.matmul(
                            aps[0:65, off + qoff : off + QB],
                            vp[t][:, 65 * h : 65 * h + 65],
                            pt[:, off + qoff : off + QB],
                            start=(t == 0),
                            stop=last,
                        )
                    prev_pv = None

                def emit_norm():
                    nonlocal pending_norm
                    if pending_norm is None:
                        return
                    aps, hA, hB, j = pending_norm
                    rrow = wp.tile([1, 2 * QB], F32R, tag="rrow", name="rrow")
                    with nc.allow_low_precision("f32r row for the broadcast matmul"):
                        nc.vector.reciprocal(rrow[:], aps[64:65, :])
                    rbp = mm_ps.tile([128, 2 * QB], F32, tag="mm", name="rbp")
                    for half2 in range(2):
                        nc.tensor.matmul(
                            rbp[0:64, QB * half2 : QB * (half2 + 1)],
                            onesr[:, 0:64],
                            rrow[:, QB * half2 : QB * (half2 + 1)],
                            start=True,
                            stop=True,
                        )
                    rb = wp.tile([64, 2 * QB], F32, tag="rb", name="rb")
                    nc.vector.tensor_copy(rb[:], rbp[0:64, :])
                    att = op.tile([64, 2 * QB], F32, tag="att_out", name="att")
                    nc.vector.tensor_mul(att[:], aps[0:64, :], rb[:])
                    for h, off in ((hA, 0), (hB, QB)):
                        nc.sync.dma_start(
                            outT[64 * h : 64 * (h + 1), QB * j : QB * (j + 1)],
                            att[:, off : off + QB],
                        )
                    pending_norm = None

                for bi, (hp, j) in enumerate(blocks):
                    hA, hB = 2 * hp, 2 * hp + 1
                    qTm, kTm = qT[hp], kTt[hp]
                    bfill = list(per_block_fillers.get(bi, ()))
                    aps = att_ps.tile([128, 2 * QB], F32, tag="att", name=f"aps{hp}_{j}")
                    for t in range(4 * j + 4):
                        i = t - 4 * j  # >= 0 only on diagonal-region tiles
                        qoff = 128 * max(i, 0)
                        qwin = slice(QB * j + qoff, QB * (j + 1))
                        ktile = slice(128 * t, 128 * (t + 1))
                        sps = mm_ps.tile([128, 2 * QB], F32, tag="mm", name="sps")
                        nc.tensor.matmul(
                            sps[:, qoff:QB],
                            kTm[0:64, ktile],
                            qTm[0:64, qwin],
                            start=True,
                            stop=True,
                            tile_position=(0, 0),
                        )
                        nc.tensor.matmul(
                            sps[:, QB + qoff : 2 * QB],
                            kTm[64:128, ktile],
                            qTm[64:128, qwin],
                            start=True,
                            stop=True,
                            tile_position=(64, 0),
                        )
                        spsv = sps[:].rearrange("p (two c) -> p two c", two=2)
                        if i >= 0:
                            nc.vector.tensor_add(
                                spsv[:, :, qoff : qoff + 128],
                                spsv[:, :, qoff : qoff + 128],
                                trid[:].rearrange("p (two c) -> p two c", two=2),
                            )
                        pt = wp.tile([128, 2 * QB], BF16, tag="pt")
                        ptv = pt[:].rearrange("p (two c) -> p two c", two=2)
                        nc.scalar.activation(
                            ptv[:, :, qoff:QB],
                            spsv[:, :, qoff:QB],
                            AF.Exp,
                            scale=float(1.0 / np.sqrt(DH)),
                        )
                        emit_pv()
                        prev_pv = (aps, hA, hB, pt, qoff, t, t == 4 * j + 3)
                        if t == 1:
                            emit_norm()
                        if t % 2 == 1 and bfill:
                            bfill.pop(0)()
                    for f in bfill:
                        f()
                    emit_pv()
                    pending_norm = (aps, hA, hB, j)
                emit_norm()

    _split_multi_waits(nc)
    return nc


def _get_runner():
    if "nc" not in _CACHE:
        _CACHE["nc"] = build_module()
    return _CACHE["nc"]


def _make_in_maps(x, Wq, bq, Wk, bk, Wv, bv):
    import ml_dtypes

    bf16 = ml_dtypes.bfloat16
    x = np.asarray(x, dtype=np.float32)
    Wq = np.asarray(Wq, dtype=bf16)
    Wk = np.asarray(Wk, dtype=bf16)
    Wv = np.asarray(Wv, dtype=bf16)
    bq = np.asarray(bq, dtype=np.float32)
    bk = np.asarray(bk, dtype=np.float32)
    bv = np.asarray(bv, dtype=np.float32)

    kp = np.arange(128)[:, None]
    qf = np.arange(128)[None, :]
    tri = np.where(kp <= qf, 0.0, NEG).astype(np.float32)
    trid = np.concatenate([tri, tri], axis=1)
    ones = np.ones((128, 1), np.float32)

    xTs = [np.ascontiguousarray(x[b].T.astype(bf16)) for b in range(B)]
    in_maps = []
    for c in range(N_CORES):
        b = c // 4
        g = c % 4
        sl = slice(DCORE * g, DCORE * (g + 1))
        in_maps.append(
            {
                "xT": xTs[b],
                "wq": np.ascontiguousarray(Wq[:, sl]),
                "wk": np.ascontiguousarray(Wk[:, sl]),
                "wv": np.ascontiguousarray(Wv[:, sl]),
                "bq": np.ascontiguousarray(bq[sl]),
                "bk": np.ascontiguousarray(bk[sl]),
                "bv": np.ascontiguousarray(bv[sl]),
                "tri": trid,
                "ones": ones,
            }
        )
    return in_maps


def kernel(x, Wq, bq, Wk, bk, Wv, bv):
    from concourse.bass_utils import run_bass_kernel_spmd

    nc = _get_runner()
    in_maps = _make_in_maps(x, Wq, bq, Wk, bk, Wv, bv)
    res = run_bass_kernel_spmd(nc, in_maps, list(range(N_CORES)))
    out = np.empty((B, S, D), dtype=np.float32)
    for c in range(N_CORES):
        b = c // 4
        g = c % 4
        out[b, :, DCORE * g : DCORE * (g + 1)] = res.results[c]["outT"].T
    return out
